# revision 29
# baseline (speedup 1.0000x reference)
"""Trainium2 Bass kernel for nn_Attention (buggy-reshape attention), 8-core SPMD.

Math (reference): q/k/v = (x @ W).reshape entangles batch and head. Each of the
256 (h,b) "chunks" is a contiguous 64-row block of the projected (16384, 512)
matrices:
  K_c = XK[64c:64c+64, :]            (64=A, 512=M)  -- used as-is
  Q_c = XQ[64c:64c+64, :].reshape(512, 64)
  V_c = XV[64c:64c+64, :].reshape(512, 64)
  out_c = softmax(Q_c @ K_c, -1) @ V_c ; final[b] = relu(mean_h out_(h,b) + x_b @ Wr)

Chunk (h,b) touches only x[4h + b//8, 64*(b%8):64*(b%8)+64, :]. We shard by
OUTPUT batch: core d owns batches 4d..4d+3 (all 8 heads) and is handed exactly
the x rows it needs -> zero collectives; head-mean is local.

Per-core layouts (m-permutation p = 64*s + r where m = 8*r + s; same perm used
for the n axis via host-permuted Wk columns):
  S^T tiles (n'-part, p-free) = Ksb_slice.T @ QTall_slice ; softmax over n'
  (partition axis; no max subtraction -- scores are O(+-50), exp fits fp32
  easily); column sums via ones-matmul; O^T = V_perm.T @ expS, normalized by
  approx-reciprocal broadcast; 1/8 head-mean folded into Wv.

v2: software-pipelined across the 16 chunk-pairs so each engine's FIFO never
head-of-line blocks: per step t -- scores(t) MMs, EXP(t) on ACT, projections
for group g+1, sum+AV(t-1) MMs, normalize(t-1) on DVE, accumulate on GPSIMD.
ACT runs EXP only; GPSIMD does the head-accumulate + final add/relu; warmup
matmuls cover the HAM ramp during input DMA.
"""

import os
import sys

import numpy as np

sys.path.insert(0, "/opt/trn_rl_repo")

import concourse.bass as bass
import concourse.bacc as bacc
import concourse.mybir as mybir
from concourse.tile import TileContext

FP = mybir.dt.float32
BF = mybir.dt.bfloat16
F16 = mybir.dt.float16
AF = mybir.ActivationFunctionType
ALU = mybir.AluOpType

B, M, E, H, A = 32, 512, 256, 8, 64
NCORES = 8

# m (and n) permutation: p = 64*s + r  <->  m = 8*r + s
_M_OF_P = np.array([8 * (p % 64) + p // 64 for p in range(512)])
_P_OF_M = np.array([64 * (m % 8) + m // 8 for m in range(512)])


def build_core_graph():
    nc = bacc.Bacc(target_bir_lowering=False)

    xaT_e = nc.declare_dram_parameter("xaT", [E, 2048], F16, isOutput=False)
    xoT_e = nc.declare_dram_parameter("xoT", [E, 2048], F16, isOutput=False)
    wqk_e = nc.declare_dram_parameter("wqk", [E, 1024], F16, isOutput=False)
    wvr_e = nc.declare_dram_parameter("wvr", [E, 512 + A], F16, isOutput=False)
    ones_e = nc.declare_dram_parameter("ones", [128, A], BF, isOutput=False)
    out_e = nc.declare_dram_parameter("out", [A, 2048], FP, isOutput=True)

    with TileContext(nc) as tc:
        from contextlib import ExitStack

        with ExitStack() as ctx:
            const = ctx.enter_context(tc.tile_pool(name="const", bufs=1))
            qt_pool = ctx.enter_context(tc.tile_pool(name="qt", bufs=2))
            ksb_pool = ctx.enter_context(tc.tile_pool(name="ksb", bufs=8))
            vsb_pool = ctx.enter_context(tc.tile_pool(name="vsb", bufs=16))
            vt_pool = ctx.enter_context(tc.tile_pool(name="vt", bufs=4))
            exps_pool = ctx.enter_context(tc.tile_pool(name="exps", bufs=4))
            misc_pool = ctx.enter_context(tc.tile_pool(name="misc", bufs=6))
            acc_pool = ctx.enter_context(tc.tile_pool(name="acc", bufs=2))

            pp_psum = ctx.enter_context(tc.tile_pool(name="pp", bufs=2, space="PSUM"))
            st_psum = ctx.enter_context(tc.tile_pool(name="st", bufs=2, space="PSUM"))
            so_psum = ctx.enter_context(tc.tile_pool(name="so", bufs=2, space="PSUM"))

            # ---- input DMAs: weights + group0 x first, then rest, then xoT ----
            xaT_t = const.tile([128, 2, 4, 4, 2, A], F16, tag="xaT")
            xoT_t = const.tile([128, 2, 2048], F16, tag="xoT")
            wqk = const.tile([128, 2, 1024], F16, tag="wqk")
            wvr = const.tile([128, 2, 512 + A], F16, tag="wvr")
            ones = const.tile([128, A], BF, tag="ones")
            # spread the input loads over several engines' DMA queues so they
            # land in parallel instead of serializing on the sync queue
            nc.sync.dma_start(out=ones[:], in_=ones_e[:, :])
            nc.sync.dma_start(
                out=wqk[:, :, :],
                in_=wqk_e[:, :].rearrange("(k p) n -> p k n", k=2))
            nc.scalar.dma_start(
                out=xaT_t[:, :, 0, :, :, :],
                in_=xaT_e[:, 0:512].rearrange("(k p) (c t r) -> p k c t r",
                                              k=2, c=4, t=2))
            nc.gpsimd.dma_start(
                out=wvr[:, :, :],
                in_=wvr_e[:, :].rearrange("(k p) n -> p k n", k=2))
            nc.gpsimd.dma_start(
                out=xaT_t[:, :, 1:4, :, :, :],
                in_=xaT_e[:, 512:2048].rearrange("(k p) (g c t r) -> p k g c t r",
                                                 k=2, g=3, c=4, t=2))
            nc.scalar.dma_start(
                out=xoT_t[:, :, :],
                in_=xoT_e[:, :].rearrange("(k p) n -> p k n", k=2))
            wq = [wqk[:, k, 0:512] for k in range(2)]
            wkp = [wqk[:, k, 512:1024] for k in range(2)]
            wv8 = [wvr[:, k, 0:512] for k in range(2)]
            wr = [wvr[:, k, 512:512 + A] for k in range(2)]
            xoT = [xoT_t[:, k, :] for k in range(2)]

            # ---- PE warmup during the DMA wait: keeps HAM at 8/8 for the
            # real matmuls and hides the clock ramp under the input load.
            warm = const.tile([128, 512], F16, tag="warm")
            nc.vector.memset(warm[:, :], 1.0)
            for w in range(8):
                pw = pp_psum.tile([128, 512], FP, tag="pp", name="pw")
                nc.tensor.matmul(pw[:, :], warm[:, 0:128], warm[:, :],
                                 start=True, stop=True)

            acc2 = [acc_pool.tile([128, 512], FP, tag="acc", name="acc")
                    for _ in range(2)]
            prt2 = const.tile([128, 2, 512], FP, tag="prt2")

            # ---- pipeline step emitters ----
            qtall = {}   # g -> tile [128, 8, 4, A]
            ksb = {}     # (g, c) -> [128, 512]
            vsbs = {}    # (g, c) -> (vsb_e, vsb_o)
            es_t = {}    # (t, par) -> [128, 4, 512] bf16

            def q_step(g, c):
                """Q^T projection for group g, s = 2c, 2c+1 (one psum bank)."""
                if c == 0:
                    qtall[g] = qt_pool.tile([128, 8, 4, A], F16, tag="qt",
                                            name="qtall")
                qp = pp_psum.tile([128, 2, 4, A], FP, tag="pp", name="qp")
                for si in range(2):
                    s = 2 * c + si
                    for k in range(2):
                        for par in range(2):
                            nc.tensor.matmul(qp[64 * par:64 * par + 64, si, :, :],
                                             wq[k][:, 64 * s:64 * s + 64],
                                             xaT_t[:, k, g, :, par, :],
                                             start=(k == 0), stop=(k == 1),
                                             skip_group_check=True)
                nc.vector.tensor_copy(qtall[g][:, 2 * c:2 * c + 2, :, :], qp[:])

            def kv_step(g, c):
                """K/V projections for chunk-pair (g,c)."""
                # K projection pair (col-packed, fp16)
                kp2 = pp_psum.tile([128, 512], FP, tag="pp", name="kp2")
                for k in range(2):
                    for par in range(2):
                        nc.tensor.matmul(kp2[64 * par:64 * par + 64, :],
                                         xaT_t[:, k, g, c, par, :], wkp[k],
                                         start=(k == 0), stop=(k == 1),
                                         skip_group_check=True)
                ksb[(g, c)] = ksb_pool.tile([128, 512], F16, tag="ksb", name="ksb")
                nc.vector.tensor_copy(ksb[(g, c)][:], kp2[:])

                # V projection pair -> V_perm tiles via tmp + shift DMAs
                pv2 = pp_psum.tile([128, 4, 2, A], FP, tag="pp", name="pv2")
                for k in range(2):
                    for par in range(2):
                        nc.tensor.matmul(
                            pv2[64 * par:64 * par + 64, :, :, :],
                            xaT_t[:, k, g, c, par, :], wv8[k],
                            start=(k == 0), stop=(k == 1),
                            skip_group_check=True)
                vsb_e = vsb_pool.tile([128, 4, A], BF, tag="vsb", name="vsbe")
                vsb_o = vsb_pool.tile([128, 4, A], BF, tag="vsb", name="vsbo")
                tmpa = vt_pool.tile([128, 4, A], BF, tag="vt", name="tmpa")
                nc.vector.tensor_copy(vsb_e[0:64, :, :], pv2[0:64, :, 0, :])
                nc.vector.tensor_copy(tmpa[0:64, :, :], pv2[0:64, :, 1, :])
                nc.vector.tensor_copy(tmpa[64:128, :, :], pv2[64:128, :, 0, :])
                nc.vector.tensor_copy(vsb_o[64:128, :, :], pv2[64:128, :, 1, :])
                nc.sync.dma_start(out=vsb_e[64:128, :, :], in_=tmpa[0:64, :, :])
                nc.sync.dma_start(out=vsb_o[0:64, :, :], in_=tmpa[64:128, :, :])
                vsbs[(g, c)] = (vsb_e, vsb_o)

            def scores_half(t, half):
                """S^T matmuls for both chunks of pair t, n-halves `half`.
                par0 uses PE rows 0-63, par1 rows 64-127; alternating them
                lets each LDWEIGHTS pull ahead under the other's stream."""
                g, c = t // 4, t % 4
                if half == 0:
                    for par in range(2):
                        es_t[(t, par)] = exps_pool.tile([128, 4, 512], BF,
                                                        tag="exps", name="es")
                st = [st_psum.tile([128, 2, 512], FP, tag="st", name="st")
                      for _ in range(2)]
                for q2 in range(2):
                    kn = 2 * half + q2
                    for par in range(2):
                        nc.tensor.matmul(
                            st[par][:, q2, :],
                            ksb[(g, c)][64 * par:64 * par + 64,
                                        128 * kn:128 * kn + 128],
                            qtall[g][64 * par:64 * par + 64, :, c, :],
                            start=True, stop=True)
                for par in range(2):
                    nc.scalar.activation(
                        es_t[(t, par)][:, 2 * half:2 * half + 2, :],
                        st[par][:], AF.Exp)

            def sumav_step(t):
                """Paired column sums + O^T for pair t; returns (sumb2, ot2)."""
                g, c = t // 4, t % 4
                sumb2 = so_psum.tile([128, 512], FP, tag="so", name="sumb2")
                for kn in range(4):
                    for par in range(2):
                        nc.tensor.matmul(sumb2[64 * par:64 * par + 64, :],
                                         ones[:, 0:A], es_t[(t, par)][:, kn, :],
                                         start=(kn == 0), stop=(kn == 3),
                                         skip_group_check=True)
                ot2 = so_psum.tile([128, 512], FP, tag="so", name="ot2")
                for kn in range(4):
                    for par in range(2):
                        nc.tensor.matmul(ot2[64 * par:64 * par + 64, :],
                                         vsbs[(g, c)][par][:, kn, :],
                                         es_t[(t, par)][:, kn, :],
                                         start=(kn == 0), stop=(kn == 3),
                                         skip_group_check=True)
                return sumb2, ot2

            def epi_step(t, sumb2, ot2):
                h, q = t // 2, t % 2
                recipb2 = misc_pool.tile([128, 512], FP, tag="recip", name="recipb2")
                nc.vector.reciprocal_approx_fast(out=recipb2[:], in_=sumb2[:])
                if h == 0:
                    nc.vector.tensor_mul(acc2[q][:], ot2[:], recipb2[:])
                elif h < 7:
                    otmp2 = misc_pool.tile([128, 512], FP, tag="otmp", name="otmp2")
                    nc.vector.tensor_mul(otmp2[:], ot2[:], recipb2[:])
                    nc.gpsimd.tensor_add(acc2[q][:], acc2[q][:], otmp2[:])
                else:
                    # last head for this q: finish on DVE/ACT and store now,
                    # split into m-halves so DVE/ACT/DMA pipeline the drain
                    outsb2 = misc_pool.tile([128, 512], FP, tag="outsb", name="outsb2")
                    for mh in range(2):
                        sl = slice(256 * mh, 256 * mh + 256)
                        otmp2 = misc_pool.tile([128, 256], FP, tag="otmp", name="otmp2")
                        nc.vector.tensor_mul(otmp2[:], ot2[:, sl], recipb2[:, sl])
                        pre2 = misc_pool.tile([128, 256], FP, tag="pre", name="pre2")
                        nc.vector.tensor_add(pre2[:], otmp2[:], acc2[q][:, sl])
                        nc.scalar.activation(outsb2[:, sl], pre2[:], AF.Relu)
                        for ph in range(2):
                            nc.sync.dma_start(
                                out=out_e[:, 512 * (2 * q + ph) + 256 * mh:
                                          512 * (2 * q + ph) + 256 * mh + 256],
                                in_=outsb2[64 * ph:64 * ph + 64, sl])

            def wr_step():
                """Wr projection (parity-paired) into prt2."""
                for q in range(2):
                    rp2 = pp_psum.tile([128, 512], FP, tag="pp", name="rp2")
                    for k in range(2):
                        for par in range(2):
                            nc.tensor.matmul(
                                rp2[64 * par:64 * par + 64, :],
                                wr[k],
                                xoT[k][:, 512 * (2 * q + par):512 * (2 * q + par) + 512],
                                start=(k == 0), stop=(k == 1),
                                skip_group_check=True)
                    nc.vector.tensor_copy(prt2[:, q, :], rp2[:])

            # ---- prologue: group-0 Q projection, 2 K/V steps ahead ----
            kv_items = [(i // 4, i % 4) for i in range(16)]
            for c in range(4):
                q_step(0, c)
            for gc in kv_items[:2]:
                kv_step(*gc)

            # ---- main pipeline over the 16 chunk-pairs; one K/V item (2-step
            # lead) and one Q item (next group) per step spread PE/DVE load.
            # sum/AV + proj matmuls fill the PE while ACT drains the EXPs. ----
            prev = None  # sum/AV + epi of pair t-1 run at step t
            for t in range(16):
                g, c = t // 4, t % 4
                scores_half(t, 0)
                scores_half(t, 1)
                if t + 2 < 16:
                    kv_step(*kv_items[t + 2])
                if g < 3:
                    q_step(g + 1, c)
                if t == 3:
                    wr_step()
                if prev is not None:
                    sumb2, ot2 = sumav_step(prev)
                    epi_step(prev, sumb2, ot2)
                if t == 4:
                    # fold the x@Wr term into both accumulators (GPSIMD idle)
                    for q in range(2):
                        nc.gpsimd.tensor_add(acc2[q][:], acc2[q][:],
                                             prt2[:, q, :])
                prev = t
            sumb2, ot2 = sumav_step(prev)
            epi_step(prev, sumb2, ot2)

    nc.finalize()
    return nc


def _stage_inputs(x, Wq, Wk, Wv, Wr):
    """Build per-core input dicts."""
    Wk_perm = np.ascontiguousarray(Wk[:, _M_OF_P].astype(np.float16))
    Wv8 = np.ascontiguousarray((Wv / 8.0).astype(np.float16))
    Wq_c = np.ascontiguousarray(Wq.astype(np.float16))
    Wr_c = np.ascontiguousarray(Wr.astype(np.float16))
    import ml_dtypes
    BF_NP = ml_dtypes.bfloat16
    in_maps = []
    for d in range(NCORES):
        xa = np.concatenate(
            [x[4 * h + d // 2, 256 * (d % 2):256 * (d % 2) + 256, :] for h in range(H)],
            axis=0)
        xaT = np.ascontiguousarray(xa.T.astype(np.float16))
        xoT = np.ascontiguousarray(
            np.concatenate([x[4 * d + i][_M_OF_P, :].T for i in range(4)],
                           axis=1).astype(np.float16))
        in_maps.append({
            "xaT": xaT, "xoT": xoT,
            "wqk": np.concatenate([Wq_c, Wk_perm], axis=1),
            "wvr": np.concatenate([Wv8, Wr_c], axis=1),
            "ones": np.ones((128, 64), BF_NP),
        })
    return in_maps


_CACHED = {}


def kernel(x, Wq, Wk, Wv, Wr, _want_trace=False):
    from concourse.bass_utils import run_bass_kernel_spmd

    x = np.asarray(x, dtype=np.float32)
    in_maps = _stage_inputs(x, np.asarray(Wq, np.float32), np.asarray(Wk, np.float32),
                            np.asarray(Wv, np.float32), np.asarray(Wr, np.float32))

    if "nc" not in _CACHED:
        _CACHED["nc"] = build_core_graph()
    nc = _CACHED["nc"]

    res = run_bass_kernel_spmd(nc, in_maps, core_ids=list(range(NCORES)),
                               trace=_want_trace)
    _CACHED["last_result"] = res

    out = np.zeros((B, M, A), np.float32)
    for d in range(NCORES):
        o = res.results[d]["out"]  # (64, 2048) = (a, 512*i + p)
        for i in range(4):
            out[4 * d + i] = o[:, 512 * i + _P_OF_M].T
    return out


if __name__ == "__main__":
    np.random.seed(0)
    pass


# revision 30
# speedup vs baseline: 1.2565x; 1.2565x over previous
"""Trainium2 Bass kernel for nn_Attention (buggy-reshape attention), 8-core SPMD.

Math (reference): q/k/v = (x @ W).reshape entangles batch and head. Each of the
256 (h,b) "chunks" is a contiguous 64-row block of the projected (16384, 512)
matrices:
  K_c = XK[64c:64c+64, :]            (64=A, 512=M)  -- used as-is
  Q_c = XQ[64c:64c+64, :].reshape(512, 64)
  V_c = XV[64c:64c+64, :].reshape(512, 64)
  out_c = softmax(Q_c @ K_c, -1) @ V_c ; final[b] = relu(mean_h out_(h,b) + x_b @ Wr)

Chunk (h,b) touches only x[4h + b//8, 64*(b%8):64*(b%8)+64, :]. We shard by
OUTPUT batch: core d owns batches 4d..4d+3 (all 8 heads) and is handed exactly
the x rows it needs -> zero collectives; head-mean is local.

Per-core layouts (m-permutation p = 64*s + r where m = 8*r + s; same perm used
for the n axis via host-permuted Wk columns):
  S^T tiles (n'-part, p-free) = Ksb_slice.T @ QTall_slice ; softmax over n'
  (partition axis; no max subtraction -- scores are O(+-50), exp fits fp32
  easily); column sums via ones-matmul; O^T = V_perm.T @ expS, normalized by
  approx-reciprocal broadcast; 1/8 head-mean folded into Wv.

v2: software-pipelined across the 16 chunk-pairs so each engine's FIFO never
head-of-line blocks: per step t -- scores(t) MMs, EXP(t) on ACT, projections
for group g+1, sum+AV(t-1) MMs, normalize(t-1) on DVE, accumulate on GPSIMD.
ACT runs EXP only; GPSIMD does the head-accumulate + final add/relu; warmup
matmuls cover the HAM ramp during input DMA.
"""

import os
import sys

import numpy as np

sys.path.insert(0, "/opt/trn_rl_repo")

import concourse.bass as bass
import concourse.bacc as bacc
import concourse.mybir as mybir
from concourse.tile import TileContext

FP = mybir.dt.float32
BF = mybir.dt.bfloat16
F16 = mybir.dt.float16
AF = mybir.ActivationFunctionType
ALU = mybir.AluOpType

B, M, E, H, A = 32, 512, 256, 8, 64
NCORES = 8

# m (and n) permutation: p = 64*s + r  <->  m = 8*r + s
_M_OF_P = np.array([8 * (p % 64) + p // 64 for p in range(512)])
_P_OF_M = np.array([64 * (m % 8) + m // 8 for m in range(512)])


def build_core_graph():
    nc = bacc.Bacc(target_bir_lowering=False)

    xaT_e = nc.declare_dram_parameter("xaT", [E, 2048], F16, isOutput=False)
    xoT_e = nc.declare_dram_parameter("xoT", [E, 2048], F16, isOutput=False)
    wqk_e = nc.declare_dram_parameter("wqk", [E, 1024], F16, isOutput=False)
    wvr_e = nc.declare_dram_parameter("wvr", [E, 512 + A], F16, isOutput=False)
    ones_e = nc.declare_dram_parameter("ones", [128, A], BF, isOutput=False)
    out_e = nc.declare_dram_parameter("out", [A, 2048], FP, isOutput=True)

    with TileContext(nc) as tc:
        from contextlib import ExitStack

        with ExitStack() as ctx:
            const = ctx.enter_context(tc.tile_pool(name="const", bufs=1))
            qt_pool = ctx.enter_context(tc.tile_pool(name="qt", bufs=2))
            ksb_pool = ctx.enter_context(tc.tile_pool(name="ksb", bufs=8))
            vsb_pool = ctx.enter_context(tc.tile_pool(name="vsb", bufs=16))
            vt_pool = ctx.enter_context(tc.tile_pool(name="vt", bufs=4))
            exps_pool = ctx.enter_context(tc.tile_pool(name="exps", bufs=4))
            misc_pool = ctx.enter_context(tc.tile_pool(name="misc", bufs=6))
            acc_pool = ctx.enter_context(tc.tile_pool(name="acc", bufs=2))

            pp_psum = ctx.enter_context(tc.tile_pool(name="pp", bufs=2, space="PSUM"))
            st_psum = ctx.enter_context(tc.tile_pool(name="st", bufs=2, space="PSUM"))
            so_psum = ctx.enter_context(tc.tile_pool(name="so", bufs=2, space="PSUM"))

            # ---- input DMAs: weights + group0 x first, then rest, then xoT ----
            xaT_t = const.tile([128, 2, 4, 4, 2, A], F16, tag="xaT")
            xoT_t = const.tile([128, 2, 2048], F16, tag="xoT")
            wqk = const.tile([128, 2, 1024], F16, tag="wqk")
            wvr = const.tile([128, 2, 512 + A], F16, tag="wvr")
            ones = const.tile([128, A], BF, tag="ones")
            # spread the input loads over several engines' DMA queues so they
            # land in parallel instead of serializing on the sync queue
            nc.sync.dma_start(out=ones[:], in_=ones_e[:, :])
            nc.sync.dma_start(
                out=wqk[:, :, :],
                in_=wqk_e[:, :].rearrange("(k p) n -> p k n", k=2))
            nc.sync.dma_start(
                out=xaT_t[:, :, 0, :, :, :],
                in_=xaT_e[:, 0:512].rearrange("(k p) (c t r) -> p k c t r",
                                              k=2, c=4, t=2))
            nc.sync.dma_start(
                out=wvr[:, :, :],
                in_=wvr_e[:, :].rearrange("(k p) n -> p k n", k=2))
            nc.sync.dma_start(
                out=xaT_t[:, :, 1:4, :, :, :],
                in_=xaT_e[:, 512:2048].rearrange("(k p) (g c t r) -> p k g c t r",
                                                 k=2, g=3, c=4, t=2))
            nc.sync.dma_start(
                out=xoT_t[:, :, :],
                in_=xoT_e[:, :].rearrange("(k p) n -> p k n", k=2))
            wq = [wqk[:, k, 0:512] for k in range(2)]
            wkp = [wqk[:, k, 512:1024] for k in range(2)]
            wv8 = [wvr[:, k, 0:512] for k in range(2)]
            wr = [wvr[:, k, 512:512 + A] for k in range(2)]
            xoT = [xoT_t[:, k, :] for k in range(2)]

            # ---- PE warmup during the DMA wait: keeps HAM at 8/8 for the
            # real matmuls and hides the clock ramp under the input load.
            warm = const.tile([128, 512], F16, tag="warm")
            nc.vector.memset(warm[:, :], 1.0)
            for w in range(8):
                pw = pp_psum.tile([128, 512], FP, tag="pp", name="pw")
                nc.tensor.matmul(pw[:, :], warm[:, 0:128], warm[:, :],
                                 start=True, stop=True)

            acc2 = [acc_pool.tile([128, 512], FP, tag="acc", name="acc")
                    for _ in range(2)]
            prt2 = const.tile([128, 2, 512], FP, tag="prt2")

            # ---- pipeline step emitters ----
            qtall = {}   # g -> tile [128, 8, 4, A]
            ksb = {}     # (g, c) -> [128, 512]
            vsbs = {}    # (g, c) -> (vsb_e, vsb_o)
            es_t = {}    # (t, par) -> [128, 4, 512] bf16

            def q_step(g, c):
                """Q^T projection for group g, s = 2c, 2c+1 (one psum bank)."""
                if c == 0:
                    qtall[g] = qt_pool.tile([128, 8, 4, A], F16, tag="qt",
                                            name="qtall")
                qp = pp_psum.tile([128, 2, 4, A], FP, tag="pp", name="qp")
                for si in range(2):
                    s = 2 * c + si
                    for k in range(2):
                        for par in range(2):
                            nc.tensor.matmul(qp[64 * par:64 * par + 64, si, :, :],
                                             wq[k][:, 64 * s:64 * s + 64],
                                             xaT_t[:, k, g, :, par, :],
                                             start=(k == 0), stop=(k == 1),
                                             skip_group_check=True)
                nc.vector.tensor_copy(qtall[g][:, 2 * c:2 * c + 2, :, :], qp[:])

            def kv_step(g, c):
                """K/V projections for chunk-pair (g,c)."""
                # K projection pair (col-packed, fp16)
                kp2 = pp_psum.tile([128, 512], FP, tag="pp", name="kp2")
                for k in range(2):
                    for par in range(2):
                        nc.tensor.matmul(kp2[64 * par:64 * par + 64, :],
                                         xaT_t[:, k, g, c, par, :], wkp[k],
                                         start=(k == 0), stop=(k == 1),
                                         skip_group_check=True)
                ksb[(g, c)] = ksb_pool.tile([128, 512], F16, tag="ksb", name="ksb")
                nc.vector.tensor_copy(ksb[(g, c)][:], kp2[:])

                # V projection pair -> V_perm tiles via tmp + shift DMAs
                pv2 = pp_psum.tile([128, 4, 2, A], FP, tag="pp", name="pv2")
                for k in range(2):
                    for par in range(2):
                        nc.tensor.matmul(
                            pv2[64 * par:64 * par + 64, :, :, :],
                            xaT_t[:, k, g, c, par, :], wv8[k],
                            start=(k == 0), stop=(k == 1),
                            skip_group_check=True)
                vsb_e = vsb_pool.tile([128, 4, A], BF, tag="vsb", name="vsbe")
                vsb_o = vsb_pool.tile([128, 4, A], BF, tag="vsb", name="vsbo")
                tmpa = vt_pool.tile([128, 4, A], BF, tag="vt", name="tmpa")
                nc.vector.tensor_copy(vsb_e[0:64, :, :], pv2[0:64, :, 0, :])
                nc.vector.tensor_copy(tmpa[0:64, :, :], pv2[0:64, :, 1, :])
                nc.vector.tensor_copy(tmpa[64:128, :, :], pv2[64:128, :, 0, :])
                nc.vector.tensor_copy(vsb_o[64:128, :, :], pv2[64:128, :, 1, :])
                nc.sync.dma_start(out=vsb_e[64:128, :, :], in_=tmpa[0:64, :, :])
                nc.sync.dma_start(out=vsb_o[0:64, :, :], in_=tmpa[64:128, :, :])
                vsbs[(g, c)] = (vsb_e, vsb_o)

            def scores_half(t, half):
                """S^T matmuls for both chunks of pair t, n-halves `half`.
                par0 uses PE rows 0-63, par1 rows 64-127; alternating them
                lets each LDWEIGHTS pull ahead under the other's stream."""
                g, c = t // 4, t % 4
                if half == 0:
                    for par in range(2):
                        es_t[(t, par)] = exps_pool.tile([128, 4, 512], BF,
                                                        tag="exps", name="es")
                st = [st_psum.tile([128, 2, 512], FP, tag="st", name="st")
                      for _ in range(2)]
                for q2 in range(2):
                    kn = 2 * half + q2
                    for par in range(2):
                        nc.tensor.matmul(
                            st[par][:, q2, :],
                            ksb[(g, c)][64 * par:64 * par + 64,
                                        128 * kn:128 * kn + 128],
                            qtall[g][64 * par:64 * par + 64, :, c, :],
                            start=True, stop=True)
                for par in range(2):
                    nc.scalar.activation(
                        es_t[(t, par)][:, 2 * half:2 * half + 2, :],
                        st[par][:], AF.Exp)

            def sumav_step(t):
                """Paired column sums + O^T for pair t; returns (sumb2, ot2)."""
                g, c = t // 4, t % 4
                sumb2 = so_psum.tile([128, 512], FP, tag="so", name="sumb2")
                for kn in range(4):
                    for par in range(2):
                        nc.tensor.matmul(sumb2[64 * par:64 * par + 64, :],
                                         ones[:, 0:A], es_t[(t, par)][:, kn, :],
                                         start=(kn == 0), stop=(kn == 3),
                                         skip_group_check=True)
                ot2 = so_psum.tile([128, 512], FP, tag="so", name="ot2")
                for kn in range(4):
                    for par in range(2):
                        nc.tensor.matmul(ot2[64 * par:64 * par + 64, :],
                                         vsbs[(g, c)][par][:, kn, :],
                                         es_t[(t, par)][:, kn, :],
                                         start=(kn == 0), stop=(kn == 3),
                                         skip_group_check=True)
                return sumb2, ot2

            def epi_step(t, sumb2, ot2):
                h, q = t // 2, t % 2
                recipb2 = misc_pool.tile([128, 512], FP, tag="recip", name="recipb2")
                nc.vector.reciprocal_approx_fast(out=recipb2[:], in_=sumb2[:])
                if h == 0:
                    nc.vector.tensor_mul(acc2[q][:], ot2[:], recipb2[:])
                elif h < 7:
                    otmp2 = misc_pool.tile([128, 512], FP, tag="otmp", name="otmp2")
                    nc.vector.tensor_mul(otmp2[:], ot2[:], recipb2[:])
                    nc.gpsimd.tensor_add(acc2[q][:], acc2[q][:], otmp2[:])
                else:
                    # last head for this q: finish on DVE/ACT and store now,
                    # split into m-halves so DVE/ACT/DMA pipeline the drain
                    outsb2 = misc_pool.tile([128, 512], FP, tag="outsb", name="outsb2")
                    for mh in range(2):
                        sl = slice(256 * mh, 256 * mh + 256)
                        otmp2 = misc_pool.tile([128, 256], FP, tag="otmp", name="otmp2")
                        nc.vector.tensor_mul(otmp2[:], ot2[:, sl], recipb2[:, sl])
                        pre2 = misc_pool.tile([128, 256], FP, tag="pre", name="pre2")
                        nc.vector.tensor_add(pre2[:], otmp2[:], acc2[q][:, sl])
                        nc.scalar.activation(outsb2[:, sl], pre2[:], AF.Relu)
                        for ph in range(2):
                            nc.sync.dma_start(
                                out=out_e[:, 512 * (2 * q + ph) + 256 * mh:
                                          512 * (2 * q + ph) + 256 * mh + 256],
                                in_=outsb2[64 * ph:64 * ph + 64, sl])

            def wr_step():
                """Wr projection (parity-paired) into prt2."""
                for q in range(2):
                    rp2 = pp_psum.tile([128, 512], FP, tag="pp", name="rp2")
                    for k in range(2):
                        for par in range(2):
                            nc.tensor.matmul(
                                rp2[64 * par:64 * par + 64, :],
                                wr[k],
                                xoT[k][:, 512 * (2 * q + par):512 * (2 * q + par) + 512],
                                start=(k == 0), stop=(k == 1),
                                skip_group_check=True)
                    nc.vector.tensor_copy(prt2[:, q, :], rp2[:])

            # ---- prologue: group-0 Q projection, 2 K/V steps ahead ----
            kv_items = [(i // 4, i % 4) for i in range(16)]
            for c in range(4):
                q_step(0, c)
            for gc in kv_items[:2]:
                kv_step(*gc)

            # ---- main pipeline over the 16 chunk-pairs; one K/V item (2-step
            # lead) and one Q item (next group) per step spread PE/DVE load.
            # sum/AV + proj matmuls fill the PE while ACT drains the EXPs. ----
            prev = None  # sum/AV + epi of pair t-1 run at step t
            for t in range(16):
                g, c = t // 4, t % 4
                scores_half(t, 0)
                scores_half(t, 1)
                if t + 2 < 16:
                    kv_step(*kv_items[t + 2])
                if g < 3:
                    q_step(g + 1, c)
                if t == 3:
                    wr_step()
                if prev is not None:
                    sumb2, ot2 = sumav_step(prev)
                    epi_step(prev, sumb2, ot2)
                if t == 4:
                    # fold the x@Wr term into both accumulators (GPSIMD idle)
                    for q in range(2):
                        nc.gpsimd.tensor_add(acc2[q][:], acc2[q][:],
                                             prt2[:, q, :])
                prev = t
            sumb2, ot2 = sumav_step(prev)
            epi_step(prev, sumb2, ot2)

    nc.finalize()
    return nc


def _stage_inputs(x, Wq, Wk, Wv, Wr):
    """Build per-core input dicts."""
    Wk_perm = np.ascontiguousarray(Wk[:, _M_OF_P].astype(np.float16))
    Wv8 = np.ascontiguousarray((Wv / 8.0).astype(np.float16))
    Wq_c = np.ascontiguousarray(Wq.astype(np.float16))
    Wr_c = np.ascontiguousarray(Wr.astype(np.float16))
    import ml_dtypes
    BF_NP = ml_dtypes.bfloat16
    in_maps = []
    for d in range(NCORES):
        xa = np.concatenate(
            [x[4 * h + d // 2, 256 * (d % 2):256 * (d % 2) + 256, :] for h in range(H)],
            axis=0)
        xaT = np.ascontiguousarray(xa.T.astype(np.float16))
        xoT = np.ascontiguousarray(
            np.concatenate([x[4 * d + i][_M_OF_P, :].T for i in range(4)],
                           axis=1).astype(np.float16))
        in_maps.append({
            "xaT": xaT, "xoT": xoT,
            "wqk": np.concatenate([Wq_c, Wk_perm], axis=1),
            "wvr": np.concatenate([Wv8, Wr_c], axis=1),
            "ones": np.ones((128, 64), BF_NP),
        })
    return in_maps


_CACHED = {}


def kernel(x, Wq, Wk, Wv, Wr, _want_trace=False):
    from concourse.bass_utils import run_bass_kernel_spmd

    x = np.asarray(x, dtype=np.float32)
    in_maps = _stage_inputs(x, np.asarray(Wq, np.float32), np.asarray(Wk, np.float32),
                            np.asarray(Wv, np.float32), np.asarray(Wr, np.float32))

    if "nc" not in _CACHED:
        _CACHED["nc"] = build_core_graph()
    nc = _CACHED["nc"]

    res = run_bass_kernel_spmd(nc, in_maps, core_ids=list(range(NCORES)),
                               trace=_want_trace)
    _CACHED["last_result"] = res

    out = np.zeros((B, M, A), np.float32)
    for d in range(NCORES):
        o = res.results[d]["out"]  # (64, 2048) = (a, 512*i + p)
        for i in range(4):
            out[4 * d + i] = o[:, 512 * i + _P_OF_M].T
    return out


if __name__ == "__main__":
    np.random.seed(0)
    pass
